# revision 4
# baseline (speedup 1.0000x reference)
"""Trainium2 Bass kernel for nn_CrossAttention_84310208020733.

Cross-attention: out = proj(softmax(mask(q @ k^T * scale)) @ v), with
  q = tgt @ q_w.T + q_b               [B=4, NT=1024, D=1024]
  k, v = split(src @ kv_w.T + kv_b)   [B=4, NS=2048, D=1024], H=16 heads, Dh=64

Sharding over 8 NeuronCores: core c handles batch b = c//2 and head group
g = c%2 (8 heads = 512 channels).  Each core computes its partial
proj-output (contraction over its 512 attn channels) in transposed layout
[out_ch, rows]; the host sums the two partials per batch, transposes, and
adds proj_b (the "all-reduce after proj" done at gather time).

On-device layout is feature-major throughout ("T" = channels on SBUF
partitions):
  qT = qwT.T @ tgtT       [512, 1024]
  kT = kwT.T @ srcT       [512, NS_kept]
  v  = srcT.T @ vwT       [NS_kept, 512]   (+ ones/zero columns for row-sums)
  sT = kT_h.T @ qT_h      [src 128, rows 512] per head pair (row-packed K=64)
  pT = exp(sT * scale + maskbias)  (ACT, bf16 out; no max-subtraction: |logits|<~4)
  av = [v_h | 1].T @ pT   -> [Dh(+1), rows] unnormalized out + row sums
  oT = av * bcast(1/sum)  [512, 1024]
  outT = pwT.T @ oT       [1024, 1024] partial, fp32

Fully-masked 128-wide src chunks (per the runtime mask, intersected across
batches) are dropped at compile time; partial masks are handled via the
additive -30000 bias inside the exp activation.
"""

import numpy as np
import ml_dtypes

import concourse.bass as bass
import concourse.bacc as bacc
import concourse.tile as tile
from concourse import mybir
from concourse.bass_utils import run_bass_kernel_spmd

P = 128
B = 4
NT = 1024
NS = 2048
D = 1024
H = 16
DH = 64
G = 2              # head groups (tensor-parallel dim)
HG = H // G        # heads per core = 8
CH = HG * DH       # channels per core = 512
KO = D // P        # 8 contraction chunks for the projections
CHO = CH // P      # 4 channel tiles per core
SCALE = DH ** -0.5
NEG = -30000.0
BF16 = mybir.dt.bfloat16
F32 = mybir.dt.float32
EXP = mybir.ActivationFunctionType.Exp
ADD = mybir.AluOpType.add

# vaug per-pair block: [A: 64 ch + 1 ones][B: 1 ones + 63 zero + 64 ch]
ABLK = DH + 1            # 65
BBLK = P                 # 128
PBLK = ABLK + BBLK       # 193


def _build_nc(nk: int) -> "bacc.Bacc":
    """Emit the per-core program for nk kept 128-wide source chunks."""
    ns_k = nk * P
    nc = bacc.Bacc("TRN2", target_bir_lowering=False, debug=False)

    tgtT = nc.dram_tensor("tgtT", [D, NT], BF16, kind="ExternalInput")
    srcT = nc.dram_tensor("srcT", [D, ns_k], BF16, kind="ExternalInput")
    qwT = nc.dram_tensor("qwT", [D, CH], BF16, kind="ExternalInput")
    kwT = nc.dram_tensor("kwT", [D, CH], BF16, kind="ExternalInput")
    vwT = nc.dram_tensor("vwT", [D, CH], BF16, kind="ExternalInput")
    pwT = nc.dram_tensor("pwT", [CH, D], BF16, kind="ExternalInput")
    qb = nc.dram_tensor("qb", [CH], F32, kind="ExternalInput")
    kb = nc.dram_tensor("kb", [CH], F32, kind="ExternalInput")
    vb = nc.dram_tensor("vb", [CH], F32, kind="ExternalInput")
    maskT = nc.dram_tensor("maskT", [P, nk], F32, kind="ExternalInput")
    outT = nc.dram_tensor("outT", [D, NT], F32, kind="ExternalOutput")

    with tile.TileContext(nc) as tc:
        with (
            tc.tile_pool(name="persist", bufs=1) as pers,
            tc.tile_pool(name="work", bufs=3) as work,
            tc.tile_pool(name="ps", bufs=2, space="PSUM") as ps,
        ):
            # ---- persistent loads -------------------------------------
            w_q = pers.tile([P, KO, CH], BF16, tag="w_q")
            nc.sync.dma_start(out=w_q[:], in_=qwT.ap().rearrange("(o p) c -> p o c", p=P))
            w_k = pers.tile([P, KO, CH], BF16, tag="w_k")
            nc.sync.dma_start(out=w_k[:], in_=kwT.ap().rearrange("(o p) c -> p o c", p=P))
            w_v = pers.tile([P, KO, CH], BF16, tag="w_v")
            nc.sync.dma_start(out=w_v[:], in_=vwT.ap().rearrange("(o p) c -> p o c", p=P))
            w_p = pers.tile([P, CHO, D], BF16, tag="w_p")
            nc.sync.dma_start(out=w_p[:], in_=pwT.ap().rearrange("(o p) c -> p o c", p=P))

            tgt_t = pers.tile([P, KO, NT], BF16, tag="tgt_t")
            nc.sync.dma_start(out=tgt_t[:], in_=tgtT.ap().rearrange("(o p) r -> p o r", p=P))
            src_t = pers.tile([P, KO, ns_k], BF16, tag="src_t")
            nc.sync.dma_start(out=src_t[:], in_=srcT.ap().rearrange("(o p) s -> p o s", p=P))

            qb_t = pers.tile([P, CHO], F32, tag="qb_t")
            nc.sync.dma_start(out=qb_t[:], in_=qb.ap().rearrange("(o p) -> p o", p=P))
            kb_t = pers.tile([P, CHO], F32, tag="kb_t")
            nc.sync.dma_start(out=kb_t[:], in_=kb.ap().rearrange("(o p) -> p o", p=P))
            # per-channel v bias broadcast across all partitions
            vb_bc = pers.tile([P, CH], F32, tag="vb_bc")
            vb_ap = vb.ap()
            vb_bcast_src = bass.AP(tensor=vb_ap.tensor, offset=vb_ap.offset,
                                   ap=[[0, P]] + list(vb_ap.ap))
            nc.gpsimd.dma_start(out=vb_bc[:], in_=vb_bcast_src)
            mask_t = pers.tile([P, nk], F32, tag="mask_t")
            nc.sync.dma_start(out=mask_t[:], in_=maskT.ap())

            qT = pers.tile([P, CHO, NT], BF16, tag="qT")
            kT = pers.tile([P, CHO, ns_k], BF16, tag="kT")
            oT = pers.tile([P, CHO, NT], BF16, tag="oT")
            vaug = [pers.tile([P, HG // 2 * PBLK], BF16, tag=f"vaug{i}", name=f"vaug{i}")
                    for i in range(nk)]

            # ---- q projection: qT[ch, rows] --------------------------
            for m in range(CHO):
                for n in range(NT // 512):
                    pmm = ps.tile([P, 512], F32, tag="mm")
                    for k in range(KO):
                        nc.tensor.matmul(
                            pmm[:], w_q[:, k, m * P:(m + 1) * P],
                            tgt_t[:, k, n * 512:(n + 1) * 512],
                            start=(k == 0), stop=(k == KO - 1))
                    nc.vector.tensor_scalar_add(
                        qT[:, m, n * 512:(n + 1) * 512], pmm[:], qb_t[:, m:m + 1])

            # ---- k projection: kT[ch, src] ---------------------------
            for m in range(CHO):
                for n in range(ns_k // 512):
                    pmm = ps.tile([P, 512], F32, tag="mm")
                    for k in range(KO):
                        nc.tensor.matmul(
                            pmm[:], w_k[:, k, m * P:(m + 1) * P],
                            src_t[:, k, n * 512:(n + 1) * 512],
                            start=(k == 0), stop=(k == KO - 1))
                    nc.vector.tensor_scalar_add(
                        kT[:, m, n * 512:(n + 1) * 512], pmm[:], kb_t[:, m:m + 1])

            # ---- v projection into packed stationary blocks ----------
            for ms in range(nk):
                pmm = ps.tile([P, 512], F32, tag="mm")
                for k in range(KO):
                    nc.tensor.matmul(
                        pmm[:], src_t[:, k, ms * P:(ms + 1) * P], w_v[:, k, :],
                        start=(k == 0), stop=(k == KO - 1))
                va = vaug[ms].rearrange("p (t c) -> p t c", c=PBLK)
                pv = pmm.rearrange("p (t c) -> p t c", c=2 * DH)
                vv = vb_bc.rearrange("p (t c) -> p t c", c=2 * DH)
                # head A data: cols 0..63 of each pair block
                nc.vector.tensor_add(va[:, :, 0:DH], pv[:, :, 0:DH], vv[:, :, 0:DH])
                # head B data: cols 129..192
                nc.vector.tensor_add(va[:, :, ABLK + DH:PBLK], pv[:, :, DH:2 * DH],
                                     vv[:, :, DH:2 * DH])
                nc.vector.memset(va[:, :, DH:DH + 1], 1.0)          # A ones col
                nc.vector.memset(va[:, :, ABLK:ABLK + 1], 1.0)      # B ones col
                nc.vector.memset(va[:, :, ABLK + 1:ABLK + DH], 0.0)  # B zero cols

            # ---- attention per head pair / row half ------------------
            for t in range(CHO):
                for n in range(2):
                    rsl = slice(n * 512, (n + 1) * 512)
                    avA = ps.tile([ABLK, 512], F32, tag="av")
                    avB = ps.tile([P, 512], F32, tag="av")
                    for j in range(nk):
                        st = ps.tile([P, 1024], F32, tag="st")
                        nc.tensor.matmul(
                            st[:, 0:512], kT[0:DH, t, j * P:(j + 1) * P],
                            qT[0:DH, t, rsl], start=True, stop=True,
                            tile_position=(0, 0))
                        nc.tensor.matmul(
                            st[:, 512:1024], kT[DH:P, t, j * P:(j + 1) * P],
                            qT[DH:P, t, rsl], start=True, stop=True,
                            tile_position=(64, 0))
                        pt = work.tile([P, 1024], BF16, tag="pt")
                        nc.scalar.activation(out=pt[:], in_=st[:], func=EXP,
                                             bias=mask_t[:, j:j + 1], scale=SCALE)
                        va = vaug[j].rearrange("p (t c) -> p t c", c=PBLK)
                        nc.tensor.matmul(avA[:], va[:, t, 0:ABLK], pt[:, 0:512],
                                         start=(j == 0), stop=(j == nk - 1))
                        nc.tensor.matmul(avB[:], va[:, t, ABLK:PBLK], pt[:, 512:1024],
                                         start=(j == 0), stop=(j == nk - 1))
                    # normalize: head A out in avA[0:64], sums in avA[64];
                    # head B out in avB[64:128], sums in avB[0].
                    recA = work.tile([P, 512], F32, tag="rec")
                    nc.vector.reciprocal(recA[DH:DH + 1, :], avA[DH:DH + 1, :])
                    # partition_broadcast sources absolute partition 0 -> bounce
                    # the reciprocal row from partition 64 to 0 via SBUF DMA
                    recAc = work.tile([1, 512], F32, tag="recAc")
                    nc.sync.dma_start(out=recAc[0:1, :], in_=recA[DH:DH + 1, :])
                    bcA = work.tile([DH, 512], F32, tag="bcA")
                    nc.gpsimd.partition_broadcast(bcA[:], recAc[0:1, :])
                    nc.vector.tensor_mul(oT[0:DH, t, rsl], avA[0:DH, :], bcA[:])

                    recB = work.tile([P, 512], F32, tag="recB")
                    nc.vector.reciprocal(recB[0:1, :], avB[0:1, :])
                    bcB = work.tile([P, 512], F32, tag="bcB")
                    nc.gpsimd.partition_broadcast(bcB[:], recB[0:1, :])
                    nc.vector.tensor_mul(oT[DH:P, t, rsl], avB[DH:P, :], bcB[DH:P, :])

            # ---- output projection (partial over this core's 512 ch) -
            for m in range(KO):
                for n in range(2):
                    pmm = ps.tile([P, 512], F32, tag="mm")
                    for k in range(CHO):
                        nc.tensor.matmul(
                            pmm[:], w_p[:, k, m * P:(m + 1) * P],
                            oT[:, k, n * 512:(n + 1) * 512],
                            start=(k == 0), stop=(k == CHO - 1))
                    ob = work.tile([P, 512], F32, tag="ob")
                    nc.vector.tensor_copy(ob[:], pmm[:])
                    nc.sync.dma_start(
                        out=outT.ap()[m * P:(m + 1) * P, n * 512:(n + 1) * 512],
                        in_=ob[:])
    nc.compile()
    return nc


_NC_CACHE: dict[int, "bacc.Bacc"] = {}


def kernel(tgt, src, src_padded_mask, q_w, q_b, kv_w, kv_b, proj_w, proj_b,
           _run_kwargs: dict | None = None):
    tgt = np.asarray(tgt, dtype=np.float32)
    src = np.asarray(src, dtype=np.float32)
    mask = np.asarray(src_padded_mask).astype(bool)
    q_w = np.asarray(q_w, dtype=np.float32)
    q_b = np.asarray(q_b, dtype=np.float32)
    kv_w = np.asarray(kv_w, dtype=np.float32)
    kv_b = np.asarray(kv_b, dtype=np.float32)
    proj_w = np.asarray(proj_w, dtype=np.float32)
    proj_b = np.asarray(proj_b, dtype=np.float32)

    # chunks of 128 src positions that are fully masked in EVERY batch can be
    # dropped at compile time; everything else is handled by the additive mask
    mchunk = mask.reshape(B, NS // P, P)
    dead = mchunk.all(axis=2).all(axis=0)            # [16]
    kept = [c for c in range(NS // P) if not dead[c]]
    if not kept:
        kept = [0]
    nk = len(kept)

    nc = _NC_CACHE.get(nk)
    if nc is None:
        nc = _build_nc(nk)
        _NC_CACHE[nk] = nc

    maskadd = np.where(mask, np.float32(NEG), np.float32(0.0)).astype(np.float32)
    bf = ml_dtypes.bfloat16

    in_maps = []
    for c in range(2 * B):
        b, g = c // 2, c % 2
        gs, ge = g * CH, (g + 1) * CH
        keep_pos = np.concatenate([np.arange(c * P, (c + 1) * P) for c in kept])
        in_maps.append({
            "tgtT": np.ascontiguousarray(tgt[b].T).astype(bf),
            "srcT": np.ascontiguousarray(src[b].T[:, keep_pos]).astype(bf),
            "qwT": np.ascontiguousarray(q_w[gs:ge].T).astype(bf),
            "kwT": np.ascontiguousarray(kv_w[gs:ge].T).astype(bf),
            "vwT": np.ascontiguousarray(kv_w[D + gs:D + ge].T).astype(bf),
            "pwT": np.ascontiguousarray(proj_w[:, gs:ge].T).astype(bf),
            "qb": q_b[gs:ge].copy(),
            "kb": kv_b[gs:ge].copy(),
            "vb": kv_b[D + gs:D + ge].copy(),
            "maskT": np.ascontiguousarray(maskadd[b][keep_pos].reshape(nk, P).T),
        })

    res = run_bass_kernel_spmd(nc, in_maps, list(range(2 * B)),
                               **(_run_kwargs or {}))
    if _run_kwargs:
        kernel.last_result = res

    out = np.empty((B, NT, D), dtype=np.float32)
    for b in range(B):
        part = res.results[2 * b]["outT"] + res.results[2 * b + 1]["outT"]
        out[b] = part.T + proj_b
    return out


# revision 6
# speedup vs baseline: 1.2552x; 1.2552x over previous
"""Trainium2 Bass kernel for nn_CrossAttention_84310208020733.

Cross-attention: out = proj(softmax(mask(q @ k^T * scale)) @ v), with
  q = tgt @ q_w.T + q_b               [B=4, NT=1024, D=1024]
  k, v = split(src @ kv_w.T + kv_b)   [B=4, NS=2048, D=1024], H=16 heads, Dh=64

Sharding over 8 NeuronCores: core c handles batch b = c//2 and head group
g = c%2 (8 heads = 512 channels).  Each core computes its partial
proj-output (contraction over its 512 attn channels) in transposed layout
[out_ch, rows]; the host sums the two partials per batch, transposes, and
adds proj_b (the "all-reduce after proj" done at gather time).

On-device layout is feature-major throughout ("T" = channels on SBUF
partitions):
  qT = qwT.T @ tgtT       [512, 1024]
  kT = kwT.T @ srcT       [512, NS_kept]
  v  = srcT.T @ vwT       [NS_kept, 512]   (+ ones/zero columns for row-sums)
  sT = kT_h.T @ qT_h      [src 128, rows 512] per head pair (row-packed K=64)
  pT = exp(sT * scale + maskbias)  (ACT, bf16 out; no max-subtraction: |logits|<~4)
  av = [v_h | 1].T @ pT   -> [Dh(+1), rows] unnormalized out + row sums
  oT = av * bcast(1/sum)  [512, 1024]
  outT = pwT.T @ oT       [1024, 1024] partial, fp32

Fully-masked 128-wide src chunks (per the runtime mask, intersected across
batches) are dropped at compile time; partial masks are handled via the
additive -30000 bias inside the exp activation.
"""

import numpy as np
import ml_dtypes

import concourse.bass as bass
import concourse.bacc as bacc
import concourse.tile as tile
from concourse import mybir
from concourse.bass_utils import run_bass_kernel_spmd

P = 128
B = 4
NT = 1024
NS = 2048
D = 1024
H = 16
DH = 64
G = 2              # head groups (tensor-parallel dim)
HG = H // G        # heads per core = 8
CH = HG * DH       # channels per core = 512
KO = D // P        # 8 contraction chunks for the projections
CHO = CH // P      # 4 channel tiles per core
SCALE = DH ** -0.5
NEG = -30000.0
BF16 = mybir.dt.bfloat16
F32 = mybir.dt.float32
EXP = mybir.ActivationFunctionType.Exp
ADD = mybir.AluOpType.add

# vaug per-pair block: [A: 64 ch + 1 ones][B: 1 ones + 63 zero + 64 ch]
ABLK = DH + 1            # 65
BBLK = P                 # 128
PBLK = ABLK + BBLK       # 193


def _build_nc(nk: int) -> "bacc.Bacc":
    """Emit the per-core program for nk kept 128-wide source chunks."""
    ns_k = nk * P
    nc = bacc.Bacc("TRN2", target_bir_lowering=False, debug=False)

    tgtT = nc.dram_tensor("tgtT", [D, NT], BF16, kind="ExternalInput")
    srcT = nc.dram_tensor("srcT", [D, ns_k], BF16, kind="ExternalInput")
    qwT = nc.dram_tensor("qwT", [D, CH], BF16, kind="ExternalInput")
    kwT = nc.dram_tensor("kwT", [D, CH], BF16, kind="ExternalInput")
    vwT = nc.dram_tensor("vwT", [D, CH], BF16, kind="ExternalInput")
    pwT = nc.dram_tensor("pwT", [CH, D], BF16, kind="ExternalInput")
    qb = nc.dram_tensor("qb", [CH], F32, kind="ExternalInput")
    kb = nc.dram_tensor("kb", [CH], F32, kind="ExternalInput")
    vb = nc.dram_tensor("vb", [CH], F32, kind="ExternalInput")
    maskT = nc.dram_tensor("maskT", [P, nk], F32, kind="ExternalInput")
    outT = nc.dram_tensor("outT", [D, NT], F32, kind="ExternalOutput")

    with tile.TileContext(nc) as tc:
        with (
            tc.tile_pool(name="persist", bufs=1) as pers,
            tc.tile_pool(name="work", bufs=3) as work,
            tc.tile_pool(name="ps", bufs=2, space="PSUM") as ps,
        ):
            # ---- persistent loads -------------------------------------
            w_q = pers.tile([P, KO, CH], BF16, tag="w_q")
            nc.sync.dma_start(out=w_q[:], in_=qwT.ap().rearrange("(o p) c -> p o c", p=P))
            w_k = pers.tile([P, KO, CH], BF16, tag="w_k")
            nc.sync.dma_start(out=w_k[:], in_=kwT.ap().rearrange("(o p) c -> p o c", p=P))
            w_v = pers.tile([P, KO, CH], BF16, tag="w_v")
            nc.sync.dma_start(out=w_v[:], in_=vwT.ap().rearrange("(o p) c -> p o c", p=P))
            w_p = pers.tile([P, CHO, D], BF16, tag="w_p")
            nc.sync.dma_start(out=w_p[:], in_=pwT.ap().rearrange("(o p) c -> p o c", p=P))

            tgt_t = pers.tile([P, KO, NT], BF16, tag="tgt_t")
            nc.sync.dma_start(out=tgt_t[:], in_=tgtT.ap().rearrange("(o p) r -> p o r", p=P))
            src_t = pers.tile([P, KO, ns_k], BF16, tag="src_t")
            nc.sync.dma_start(out=src_t[:], in_=srcT.ap().rearrange("(o p) s -> p o s", p=P))

            qb_t = pers.tile([P, CHO], F32, tag="qb_t")
            nc.sync.dma_start(out=qb_t[:], in_=qb.ap().rearrange("(o p) -> p o", p=P))
            kb_t = pers.tile([P, CHO], F32, tag="kb_t")
            nc.sync.dma_start(out=kb_t[:], in_=kb.ap().rearrange("(o p) -> p o", p=P))
            # per-channel v bias broadcast across all partitions
            vb_bc = pers.tile([P, CH], F32, tag="vb_bc")
            vb_ap = vb.ap()
            vb_bcast_src = bass.AP(tensor=vb_ap.tensor, offset=vb_ap.offset,
                                   ap=[[0, P]] + list(vb_ap.ap))
            nc.gpsimd.dma_start(out=vb_bc[:], in_=vb_bcast_src)
            mask_t = pers.tile([P, nk], F32, tag="mask_t")
            nc.sync.dma_start(out=mask_t[:], in_=maskT.ap())

            ones_t = pers.tile([P, P], BF16, tag="ones_t")
            nc.vector.memset(ones_t[:], 1.0)
            qT = pers.tile([P, CHO, NT], BF16, tag="qT")
            kT = pers.tile([P, CHO, ns_k], BF16, tag="kT")
            oT = pers.tile([P, CHO, NT], BF16, tag="oT")
            vaug = [pers.tile([P, HG // 2 * PBLK], BF16, tag=f"vaug{i}", name=f"vaug{i}")
                    for i in range(nk)]

            # ---- projections + attention, interleaved ---------------
            # Emission order: QT(m=0), KT(m=0), V(all), pair0, QT(1), KT(1),
            # pair1, ... keeps PE fed while ACT churns on exp (the attention
            # window is ACT-bound), and keeps HAM warm.
            def qt_group(m):
                for n in range(NT // 512):
                    pmm = ps.tile([P, 512], F32, tag="acc", bufs=4, name="pmm_q")
                    for k in range(KO):
                        nc.tensor.matmul(
                            pmm[:], w_q[:, k, m * P:(m + 1) * P],
                            tgt_t[:, k, n * 512:(n + 1) * 512],
                            start=(k == 0), stop=(k == KO - 1))
                    nc.vector.tensor_scalar_add(
                        qT[:, m, n * 512:(n + 1) * 512], pmm[:], qb_t[:, m:m + 1])

            def kt_group(m):
                for n in range(ns_k // 512):
                    pmm = ps.tile([P, 512], F32, tag="acc", bufs=4, name="pmm_k")
                    for k in range(KO):
                        nc.tensor.matmul(
                            pmm[:], w_k[:, k, m * P:(m + 1) * P],
                            src_t[:, k, n * 512:(n + 1) * 512],
                            start=(k == 0), stop=(k == KO - 1))
                    nc.vector.tensor_scalar_add(
                        kT[:, m, n * 512:(n + 1) * 512], pmm[:], kb_t[:, m:m + 1])

            def v_group(ms):
                pmm = ps.tile([P, 512], F32, tag="acc", bufs=4, name="pmm_v")
                for k in range(KO):
                    nc.tensor.matmul(
                        pmm[:], src_t[:, k, ms * P:(ms + 1) * P], w_v[:, k, :],
                        start=(k == 0), stop=(k == KO - 1))
                va = vaug[ms].rearrange("p (t c) -> p t c", c=PBLK)
                pv = pmm.rearrange("p (t c) -> p t c", c=2 * DH)
                vv = vb_bc.rearrange("p (t c) -> p t c", c=2 * DH)
                nc.vector.tensor_add(va[:, :, 0:DH], pv[:, :, 0:DH], vv[:, :, 0:DH])
                nc.vector.tensor_add(va[:, :, ABLK + DH:PBLK], pv[:, :, DH:2 * DH],
                                     vv[:, :, DH:2 * DH])
                nc.vector.memset(va[:, :, DH:DH + 1], 1.0)
                nc.vector.memset(va[:, :, ABLK:ABLK + 1], 1.0)
                nc.vector.memset(va[:, :, ABLK + 1:ABLK + DH], 0.0)

            def attn_pair(t):
                for n in range(2):
                    rsl = slice(n * 512, (n + 1) * 512)
                    avA = ps.tile([ABLK, 512], F32, tag="acc", bufs=4, name="avA")
                    avB = ps.tile([P, 512], F32, tag="acc", bufs=4, name="avB")
                    for j in range(nk):
                        st = ps.tile([P, 1024], F32, tag="st", name="st")
                        nc.tensor.matmul(
                            st[:, 0:512], kT[0:DH, t, j * P:(j + 1) * P],
                            qT[0:DH, t, rsl], start=True, stop=True,
                            tile_position=(0, 0))
                        nc.tensor.matmul(
                            st[:, 512:1024], kT[DH:P, t, j * P:(j + 1) * P],
                            qT[DH:P, t, rsl], start=True, stop=True,
                            tile_position=(64, 0))
                        pt = work.tile([P, 1024], BF16, tag="pt", name="pt")
                        nc.scalar.activation(out=pt[:], in_=st[:], func=EXP,
                                             bias=mask_t[:, j:j + 1], scale=SCALE)
                        va = vaug[j].rearrange("p (t c) -> p t c", c=PBLK)
                        nc.tensor.matmul(avA[:], va[:, t, 0:ABLK], pt[:, 0:512],
                                         start=(j == 0), stop=(j == nk - 1))
                        nc.tensor.matmul(avB[:], va[:, t, ABLK:PBLK], pt[:, 512:1024],
                                         start=(j == 0), stop=(j == nk - 1))
                    # normalization: wide ops only (single-partition DVE is
                    # both slow and broken for the custom recip).  Copy av to
                    # SBUF (bf16), broadcast the sums row with a K=1 ones
                    # matmul, wide approx-reciprocal, wide multiply.
                    avAs = work.tile([ABLK, 512], BF16, tag="avAs")
                    nc.vector.tensor_copy(avAs[:], avA[:])
                    avBs = work.tile([P, 512], BF16, tag="avBs")
                    nc.vector.tensor_copy(avBs[:], avB[:])
                    bsA = ps.tile([P, 512], F32, tag="acc", bufs=4, name="bsA")
                    nc.tensor.matmul(bsA[:], ones_t[DH:DH + 1, :], avAs[DH:DH + 1, :],
                                     start=True, stop=True)
                    rbA = work.tile([P, 512], F32, tag="rbA")
                    nc.vector.reciprocal_approx_fast(rbA[:], bsA[:])
                    nc.vector.tensor_mul(oT[0:DH, t, rsl], avAs[0:DH, :], rbA[0:DH, :])
                    bsB = ps.tile([P, 512], F32, tag="acc", bufs=4, name="bsB")
                    nc.tensor.matmul(bsB[:], ones_t[0:1, :], avBs[0:1, :],
                                     start=True, stop=True)
                    rbB = work.tile([P, 512], F32, tag="rbB")
                    nc.vector.reciprocal_approx_fast(rbB[:], bsB[:])
                    nc.vector.tensor_mul(oT[DH:P, t, rsl], avBs[DH:P, :], rbB[DH:P, :])

            qt_group(0)
            kt_group(0)
            for ms in range(nk):
                v_group(ms)
            attn_pair(0)
            for t in range(1, CHO):
                qt_group(t)
                kt_group(t)
                attn_pair(t)

            # ---- output projection (partial over this core's 512 ch) -
            for m in range(KO):
                for n in range(2):
                    pmm = ps.tile([P, 512], F32, tag="acc", bufs=4, name="pmm_p")
                    for k in range(CHO):
                        nc.tensor.matmul(
                            pmm[:], w_p[:, k, m * P:(m + 1) * P],
                            oT[:, k, n * 512:(n + 1) * 512],
                            start=(k == 0), stop=(k == CHO - 1))
                    ob = work.tile([P, 512], F32, tag="ob")
                    nc.vector.tensor_copy(ob[:], pmm[:])
                    nc.sync.dma_start(
                        out=outT.ap()[m * P:(m + 1) * P, n * 512:(n + 1) * 512],
                        in_=ob[:])
    nc.compile()
    return nc


_NC_CACHE: dict[int, "bacc.Bacc"] = {}


def kernel(tgt, src, src_padded_mask, q_w, q_b, kv_w, kv_b, proj_w, proj_b,
           _run_kwargs: dict | None = None):
    tgt = np.asarray(tgt, dtype=np.float32)
    src = np.asarray(src, dtype=np.float32)
    mask = np.asarray(src_padded_mask).astype(bool)
    q_w = np.asarray(q_w, dtype=np.float32)
    q_b = np.asarray(q_b, dtype=np.float32)
    kv_w = np.asarray(kv_w, dtype=np.float32)
    kv_b = np.asarray(kv_b, dtype=np.float32)
    proj_w = np.asarray(proj_w, dtype=np.float32)
    proj_b = np.asarray(proj_b, dtype=np.float32)

    # chunks of 128 src positions that are fully masked in EVERY batch can be
    # dropped at compile time; everything else is handled by the additive mask
    mchunk = mask.reshape(B, NS // P, P)
    dead = mchunk.all(axis=2).all(axis=0)            # [16]
    kept = [c for c in range(NS // P) if not dead[c]]
    if not kept:
        kept = [0]
    nk = len(kept)

    nc = _NC_CACHE.get(nk)
    if nc is None:
        nc = _build_nc(nk)
        _NC_CACHE[nk] = nc

    maskadd = np.where(mask, np.float32(NEG), np.float32(0.0)).astype(np.float32)
    bf = ml_dtypes.bfloat16

    in_maps = []
    for c in range(2 * B):
        b, g = c // 2, c % 2
        gs, ge = g * CH, (g + 1) * CH
        keep_pos = np.concatenate([np.arange(c * P, (c + 1) * P) for c in kept])
        in_maps.append({
            "tgtT": np.ascontiguousarray(tgt[b].T).astype(bf),
            "srcT": np.ascontiguousarray(src[b].T[:, keep_pos]).astype(bf),
            "qwT": np.ascontiguousarray(q_w[gs:ge].T).astype(bf),
            "kwT": np.ascontiguousarray(kv_w[gs:ge].T).astype(bf),
            "vwT": np.ascontiguousarray(kv_w[D + gs:D + ge].T).astype(bf),
            "pwT": np.ascontiguousarray(proj_w[:, gs:ge].T).astype(bf),
            "qb": q_b[gs:ge].copy(),
            "kb": kv_b[gs:ge].copy(),
            "vb": kv_b[D + gs:D + ge].copy(),
            "maskT": np.ascontiguousarray(maskadd[b][keep_pos].reshape(nk, P).T),
        })

    res = run_bass_kernel_spmd(nc, in_maps, list(range(2 * B)),
                               **(_run_kwargs or {}))
    if _run_kwargs:
        kernel.last_result = res

    out = np.empty((B, NT, D), dtype=np.float32)
    for b in range(B):
        part = res.results[2 * b]["outT"] + res.results[2 * b + 1]["outT"]
        out[b] = part.T + proj_b
    return out


# revision 8
# speedup vs baseline: 1.3023x; 1.0375x over previous
"""Trainium2 Bass kernel for nn_CrossAttention_84310208020733.

Cross-attention: out = proj(softmax(mask(q @ k^T * scale)) @ v), with
  q = tgt @ q_w.T + q_b               [B=4, NT=1024, D=1024]
  k, v = split(src @ kv_w.T + kv_b)   [B=4, NS=2048, D=1024], H=16 heads, Dh=64

Sharding over 8 NeuronCores: core c handles batch b = c//2 and head group
g = c%2 (8 heads = 512 channels).  Each core computes its partial
proj-output (contraction over its 512 attn channels) in transposed layout
[out_ch, rows]; the host sums the two partials per batch, transposes, and
adds proj_b (the "all-reduce after proj" done at gather time).

On-device layout is feature-major throughout ("T" = channels on SBUF
partitions):
  qT = qwT.T @ tgtT       [512, 1024]
  kT = kwT.T @ srcT       [512, NS_kept]
  v  = srcT.T @ vwT       [NS_kept, 512]   (+ ones/zero columns for row-sums)
  sT = kT_h.T @ qT_h      [src 128, rows 512] per head pair (row-packed K=64)
  pT = exp(sT * scale + maskbias)  (ACT, bf16 out; no max-subtraction: |logits|<~4)
  av = [v_h | 1].T @ pT   -> [Dh(+1), rows] unnormalized out + row sums
  oT = av * bcast(1/sum)  [512, 1024]
  outT = pwT.T @ oT       [1024, 1024] partial, fp32

Fully-masked 128-wide src chunks (per the runtime mask, intersected across
batches) are dropped at compile time; partial masks are handled via the
additive -30000 bias inside the exp activation.
"""

import numpy as np
import ml_dtypes

import concourse.bass as bass
import concourse.bacc as bacc
import concourse.tile as tile
from concourse import mybir
from concourse.bass_utils import run_bass_kernel_spmd

P = 128
B = 4
NT = 1024
NS = 2048
D = 1024
H = 16
DH = 64
G = 2              # head groups (tensor-parallel dim)
HG = H // G        # heads per core = 8
CH = HG * DH       # channels per core = 512
KO = D // P        # 8 contraction chunks for the projections
CHO = CH // P      # 4 channel tiles per core
SCALE = DH ** -0.5
NEG = -30000.0
BF16 = mybir.dt.bfloat16
F32 = mybir.dt.float32
EXP = mybir.ActivationFunctionType.Exp
ADD = mybir.AluOpType.add

# vaug per-pair block: [A: 64 ch + 1 ones][B: 1 ones + 63 zero + 64 ch]
ABLK = DH + 1            # 65
BBLK = P                 # 128
PBLK = ABLK + BBLK       # 193


def _build_nc(nk: int) -> "bacc.Bacc":
    """Emit the per-core program for nk kept 128-wide source chunks."""
    ns_k = nk * P
    nc = bacc.Bacc("TRN2", target_bir_lowering=False, debug=False)

    tgtT = nc.dram_tensor("tgtT", [D, NT], BF16, kind="ExternalInput")
    srcT = nc.dram_tensor("srcT", [D, ns_k], BF16, kind="ExternalInput")
    qwT = nc.dram_tensor("qwT", [D, CH], BF16, kind="ExternalInput")
    kwT = nc.dram_tensor("kwT", [D, CH], BF16, kind="ExternalInput")
    vwT = nc.dram_tensor("vwT", [D, CH], BF16, kind="ExternalInput")
    pwT = nc.dram_tensor("pwT", [CH, D], BF16, kind="ExternalInput")
    qb = nc.dram_tensor("qb", [CH], F32, kind="ExternalInput")
    kb = nc.dram_tensor("kb", [CH], F32, kind="ExternalInput")
    vb = nc.dram_tensor("vb", [CH], F32, kind="ExternalInput")
    maskT = nc.dram_tensor("maskT", [P, nk], F32, kind="ExternalInput")
    outT = nc.dram_tensor("outT", [D, NT], F32, kind="ExternalOutput")

    with tile.TileContext(nc) as tc:
        with (
            tc.tile_pool(name="persist", bufs=1) as pers,
            tc.tile_pool(name="work", bufs=3) as work,
            tc.tile_pool(name="ps", bufs=2, space="PSUM") as ps,
        ):
            # ---- persistent loads, split per contraction chunk so the
            # first matmuls wait on ~KB-scale DMAs, not MB-scale ones ----
            mask_t = pers.tile([P, nk], F32, tag="mask_t")
            nc.sync.dma_start(out=mask_t[:], in_=maskT.ap())
            qb_t = pers.tile([P, CHO], F32, tag="qb_t")
            nc.sync.dma_start(out=qb_t[:], in_=qb.ap().rearrange("(o p) -> p o", p=P))
            kb_t = pers.tile([P, CHO], F32, tag="kb_t")
            nc.sync.dma_start(out=kb_t[:], in_=kb.ap().rearrange("(o p) -> p o", p=P))
            vb_bc = pers.tile([P, CH], F32, tag="vb_bc")
            vb_ap = vb.ap()
            vb_bcast_src = bass.AP(tensor=vb_ap.tensor, offset=vb_ap.offset,
                                   ap=[[0, P]] + list(vb_ap.ap))
            nc.gpsimd.dma_start(out=vb_bc[:], in_=vb_bcast_src)

            def kslices(dram, width, tagp):
                tiles = []
                for k in range(KO):
                    tl = pers.tile([P, width], BF16, tag=f"{tagp}{k}", name=f"{tagp}{k}")
                    nc.sync.dma_start(out=tl[:], in_=dram.ap()[k * P:(k + 1) * P, :])
                    tiles.append(tl)
                return tiles

            w_q = kslices(qwT, CH, "w_q")
            tgt_t = kslices(tgtT, NT, "tgt")
            w_k = kslices(kwT, CH, "w_k")
            src_t = kslices(srcT, ns_k, "src")
            w_v = kslices(vwT, CH, "w_v")
            w_p = pers.tile([P, CHO, D], BF16, tag="w_p")
            nc.sync.dma_start(out=w_p[:], in_=pwT.ap().rearrange("(o p) c -> p o c", p=P))

            ones_t = pers.tile([P, P], BF16, tag="ones_t")
            nc.vector.memset(ones_t[:], 1.0)
            qT = pers.tile([P, CHO, NT], BF16, tag="qT")
            kT = pers.tile([P, CHO, ns_k], BF16, tag="kT")
            oT = pers.tile([P, CHO, NT], BF16, tag="oT")
            vaug = [pers.tile([P, HG // 2 * PBLK], BF16, tag=f"vaug{i}", name=f"vaug{i}")
                    for i in range(nk)]

            # ---- emission units --------------------------------------
            def qt_group(m, n):
                pmm = ps.tile([P, 512], F32, tag="acc", bufs=4, name="pmm_q")
                for k in range(KO):
                    nc.tensor.matmul(
                        pmm[:], w_q[k][:, m * P:(m + 1) * P],
                        tgt_t[k][:, n * 512:(n + 1) * 512],
                        start=(k == 0), stop=(k == KO - 1))
                nc.vector.tensor_scalar_add(
                    qT[:, m, n * 512:(n + 1) * 512], pmm[:], qb_t[:, m:m + 1])

            def kt_group(m, n):
                pmm = ps.tile([P, 512], F32, tag="acc", bufs=4, name="pmm_k")
                for k in range(KO):
                    nc.tensor.matmul(
                        pmm[:], w_k[k][:, m * P:(m + 1) * P],
                        src_t[k][:, n * 512:(n + 1) * 512],
                        start=(k == 0), stop=(k == KO - 1))
                nc.vector.tensor_scalar_add(
                    kT[:, m, n * 512:(n + 1) * 512], pmm[:], kb_t[:, m:m + 1])

            def v_group(ms):
                pmm = ps.tile([P, 512], F32, tag="acc", bufs=4, name="pmm_v")
                for k in range(KO):
                    nc.tensor.matmul(
                        pmm[:], src_t[k][:, ms * P:(ms + 1) * P], w_v[k][:],
                        start=(k == 0), stop=(k == KO - 1))
                va = vaug[ms].rearrange("p (t c) -> p t c", c=PBLK)
                pv = pmm.rearrange("p (t c) -> p t c", c=2 * DH)
                vv = vb_bc.rearrange("p (t c) -> p t c", c=2 * DH)
                nc.vector.tensor_add(va[:, :, 0:DH], pv[:, :, 0:DH], vv[:, :, 0:DH])
                nc.vector.tensor_add(va[:, :, ABLK + DH:PBLK], pv[:, :, DH:2 * DH],
                                     vv[:, :, DH:2 * DH])
                nc.vector.memset(va[:, :, DH:DH + 1], 1.0)
                nc.vector.memset(va[:, :, ABLK:ABLK + 1], 1.0)
                nc.vector.memset(va[:, :, ABLK + 1:ABLK + DH], 0.0)

            def proj_group(m, n):
                pmm = ps.tile([P, 512], F32, tag="acc", bufs=4, name="pmm_p")
                for k in range(CHO):
                    nc.tensor.matmul(
                        pmm[:], w_p[:, k, m * P:(m + 1) * P],
                        oT[:, k, n * 512:(n + 1) * 512],
                        start=(k == 0), stop=(k == CHO - 1))
                ob = work.tile([P, 512], F32, tag="ob")
                nc.vector.tensor_copy(ob[:], pmm[:])
                nc.sync.dma_start(
                    out=outT.ap()[m * P:(m + 1) * P, n * 512:(n + 1) * 512],
                    in_=ob[:])

            # filler queues: PE work woven between ACT-bound attention
            # chunks.  "crit" units must complete before the next block
            # starts (force-drained at block boundaries); "lazy" units have
            # distant consumers.
            crit = []
            lazy = []

            def drain(nu):
                for _ in range(nu):
                    if crit:
                        crit.pop(0)()
                    elif lazy:
                        lazy.pop(0)()

            def attn_block(t, n, pre_chunk=None):
                rsl = slice(n * 512, (n + 1) * 512)
                avA = ps.tile([ABLK, 512], F32, tag="acc", bufs=4, name="avA")
                avB = ps.tile([P, 512], F32, tag="acc", bufs=4, name="avB")
                for j in range(nk):
                    if pre_chunk is not None:
                        pre_chunk(j)
                    elif j % 2 == 1:
                        drain(1)
                    st = ps.tile([P, 1024], F32, tag="st", name="st")
                    nc.tensor.matmul(
                        st[:, 0:512], kT[0:DH, t, j * P:(j + 1) * P],
                        qT[0:DH, t, rsl], start=True, stop=True,
                        tile_position=(0, 0))
                    nc.tensor.matmul(
                        st[:, 512:1024], kT[DH:P, t, j * P:(j + 1) * P],
                        qT[DH:P, t, rsl], start=True, stop=True,
                        tile_position=(64, 0))
                    pt = work.tile([P, 1024], BF16, tag="pt", name="pt")
                    nc.scalar.activation(out=pt[:], in_=st[:], func=EXP,
                                         bias=mask_t[:, j:j + 1], scale=SCALE)
                    va = vaug[j].rearrange("p (t c) -> p t c", c=PBLK)
                    nc.tensor.matmul(avA[:], va[:, t, 0:ABLK], pt[:, 0:512],
                                     start=(j == 0), stop=(j == nk - 1))
                    nc.tensor.matmul(avB[:], va[:, t, ABLK:PBLK], pt[:, 512:1024],
                                     start=(j == 0), stop=(j == nk - 1))
                # normalization, wide ops only
                avAs = work.tile([ABLK, 512], BF16, tag="avAs")
                nc.vector.tensor_copy(avAs[:], avA[:])
                avBs = work.tile([P, 512], BF16, tag="avBs")
                nc.vector.tensor_copy(avBs[:], avB[:])
                bsA = ps.tile([P, 512], F32, tag="acc", bufs=4, name="bsA")
                nc.tensor.matmul(bsA[:], ones_t[DH:DH + 1, :], avAs[DH:DH + 1, :],
                                 start=True, stop=True)
                rbA = work.tile([P, 512], F32, tag="rbA")
                nc.vector.reciprocal_approx_fast(rbA[:], bsA[:])
                nc.vector.tensor_mul(oT[0:DH, t, rsl], avAs[0:DH, :], rbA[0:DH, :])
                bsB = ps.tile([P, 512], F32, tag="acc", bufs=4, name="bsB")
                nc.tensor.matmul(bsB[:], ones_t[0:1, :], avBs[0:1, :],
                                 start=True, stop=True)
                rbB = work.tile([P, 512], F32, tag="rbB")
                nc.vector.reciprocal_approx_fast(rbB[:], bsB[:])
                nc.vector.tensor_mul(oT[DH:P, t, rsl], avBs[DH:P, :], rbB[DH:P, :])

            # ---- schedule --------------------------------------------
            # n=0 sweep: pair t's kt/qt groups are woven (as "critical"
            # filler) into pair t-1's ACT-bound chunk loop and force-drained
            # at the block boundary; v-projection is woven into block (0,0).
            # n=1 sweep: weave the n=0 half of the output projection.
            NSB = ns_k // 512

            def mk(f, *a):
                return lambda: f(*a)

            qt1_done = [False] * CHO

            def qt1(t):
                if not qt1_done[t]:
                    qt1_done[t] = True
                    qt_group(t, 1)

            qt_group(0, 0)
            for x in range(NSB):
                kt_group(0, x)

            for t in range(CHO):
                if t + 1 < CHO:
                    crit.extend([mk(qt_group, t + 1, 0)]
                                + [mk(kt_group, t + 1, x) for x in range(NSB)])
                lazy.append(mk(qt1, t))
                if t == 0:
                    attn_block(0, 0, pre_chunk=lambda j: v_group(j))
                else:
                    attn_block(t, 0)
                while crit:
                    crit.pop(0)()

            lazy.extend([mk(proj_group, m, 0) for m in range(KO)])
            for t in range(CHO):
                qt1(t)
                attn_block(t, 1)
            while lazy:
                lazy.pop(0)()
            for m in range(KO):
                proj_group(m, 1)
    nc.compile()
    return nc


_NC_CACHE: dict[int, "bacc.Bacc"] = {}


def kernel(tgt, src, src_padded_mask, q_w, q_b, kv_w, kv_b, proj_w, proj_b,
           _run_kwargs: dict | None = None):
    tgt = np.asarray(tgt, dtype=np.float32)
    src = np.asarray(src, dtype=np.float32)
    mask = np.asarray(src_padded_mask).astype(bool)
    q_w = np.asarray(q_w, dtype=np.float32)
    q_b = np.asarray(q_b, dtype=np.float32)
    kv_w = np.asarray(kv_w, dtype=np.float32)
    kv_b = np.asarray(kv_b, dtype=np.float32)
    proj_w = np.asarray(proj_w, dtype=np.float32)
    proj_b = np.asarray(proj_b, dtype=np.float32)

    # chunks of 128 src positions that are fully masked in EVERY batch can be
    # dropped at compile time; everything else is handled by the additive mask
    mchunk = mask.reshape(B, NS // P, P)
    dead = mchunk.all(axis=2).all(axis=0)            # [16]
    kept = [c for c in range(NS // P) if not dead[c]]
    if not kept:
        kept = [0]
    nk = len(kept)

    nc = _NC_CACHE.get(nk)
    if nc is None:
        nc = _build_nc(nk)
        _NC_CACHE[nk] = nc

    maskadd = np.where(mask, np.float32(NEG), np.float32(0.0)).astype(np.float32)
    bf = ml_dtypes.bfloat16

    in_maps = []
    for c in range(2 * B):
        b, g = c // 2, c % 2
        gs, ge = g * CH, (g + 1) * CH
        keep_pos = np.concatenate([np.arange(c * P, (c + 1) * P) for c in kept])
        in_maps.append({
            "tgtT": np.ascontiguousarray(tgt[b].T).astype(bf),
            "srcT": np.ascontiguousarray(src[b].T[:, keep_pos]).astype(bf),
            "qwT": np.ascontiguousarray(q_w[gs:ge].T).astype(bf),
            "kwT": np.ascontiguousarray(kv_w[gs:ge].T).astype(bf),
            "vwT": np.ascontiguousarray(kv_w[D + gs:D + ge].T).astype(bf),
            "pwT": np.ascontiguousarray(proj_w[:, gs:ge].T).astype(bf),
            "qb": q_b[gs:ge].copy(),
            "kb": kv_b[gs:ge].copy(),
            "vb": kv_b[D + gs:D + ge].copy(),
            "maskT": np.ascontiguousarray(maskadd[b][keep_pos].reshape(nk, P).T),
        })

    res = run_bass_kernel_spmd(nc, in_maps, list(range(2 * B)),
                               **(_run_kwargs or {}))
    if _run_kwargs:
        kernel.last_result = res

    out = np.empty((B, NT, D), dtype=np.float32)
    for b in range(B):
        part = res.results[2 * b]["outT"] + res.results[2 * b + 1]["outT"]
        out[b] = part.T + proj_b
    return out
